# revision 36
# baseline (speedup 1.0000x reference)
"""DeepseekV4 MLP (fp8-block-quantized gate_up/down with qdq activations and
clamped SwiGLU) on 8 Trainium2 NeuronCores.

Strategy: data-parallel over tokens (512 tokens/core), full weights streamed
per core, all matmuls in fp8 with perf_mode=DoubleRow (K=256 per matmul,
~1.5x bf16 PE throughput).

Numerics: every scale in the reference (weight block scales and activation
qdq scales) is a power of two, and fp8 e4m3 rounding is relative, so
power-of-2 scaling commutes with the quantization rounding:
  - weight dequant wq*s fits fp8 after one GLOBAL power-of-2 fold 2^E
    (host-side, exact except a negligible sub-denormal tail), and
  - the reference's per-block activation qdq  round(x/s)*s  equals a raw
    fp8 cast of x (values stay far below the TRN float8e4 max of 240),
    again up to the denormal tail.
So the device computes: cast x to fp8, fp8 matmul w_folded, multiply PSUM by
2^-E on eviction, swiglu in f32, cast h to fp8, fp8 matmul down, scale, out.
The 2^-E constants arrive via a tiny "cst" input so E stays data-driven.

Matmul orientation: weights are the stationary operand [k,128 cols] (fresh
per matmul; LDWEIGHTS hides behind the previous matmul via the PE background
weight buffer), activations stream as the moving operand [k, 512 tokens].
Outputs land [cols, tokens] in PSUM: gate_up output tiles ARE the down
matmul's moving-operand layout, so no on-device transpose anywhere. The down
output [hcol, token] is transposed on the host during unshard.
"""

import numpy as np

import concourse.bass as bass
import concourse.mybir as mybir
import concourse.tile as tile
from concourse import bass_utils

F32 = mybir.dt.float32
FP8 = mybir.dt.float8e4
NP_FP8 = mybir.dt.np(FP8)  # ml_dtypes.float8_e4m3 (max 240 == TRN float8e4)
AF = mybir.ActivationFunctionType
ALU = mybir.AluOpType
DR = mybir.MatmulPerfMode.DoubleRow

T, H, I = 4096, 4096, 11008
N_CORES = 8
TC = T // N_CORES            # 512 tokens per core
LIMIT = 7.0

NIB = I // 128               # 86 gate (and up) column blocks
KBA = H // 128               # 32 contraction blocks, gate_up
DKA = KBA // 2               # 16 double-k pairs
DKB = NIB // 2               # 43 double-k pairs, down
NHB = H // 128               # 32 down output column blocks


def build_nc(tc_tokens=TC, h=H, i_dim=I, waitfix=True, use_fp8=True):
    """Per-core Bass program. DRAM tensors:
      xt  [128, 32, 512] f32   x transposed: [k_in_block, kb, token]
      wa  [86, 2, 128, 32, 128] fp8  gate_up folded: [iblk, gate/up, k_in, kb, col]
      wb  [32, 128, 86, 128] fp8     down folded: [hblk, k_in, kb, col]
      cst [128, 8] f32         per-partition broadcast constants
      out [32, 128, 512] f32   down output: [hblk, hcol, token]
    """
    assert use_fp8
    nib = i_dim // 128
    kba = h // 128
    dka = kba // 2
    dkb = nib // 2
    nhb = h // 128

    nc = bass.Bass("TRN2", target_bir_lowering=False, debug=False, num_devices=1)
    xt_d = nc.dram_tensor("xt", [128, kba, tc_tokens], F32, kind="ExternalInput")
    wa_d = nc.dram_tensor("wa", [nib, 2, 128, kba, 128], FP8, kind="ExternalInput")
    wb_d = nc.dram_tensor("wb", [nhb, 128, nib, 128], FP8, kind="ExternalInput")
    cst_d = nc.dram_tensor("cst", [128, 8], F32, kind="ExternalInput")
    out_d = nc.dram_tensor("out", [nhb, 128, tc_tokens], F32, kind="ExternalOutput")

    with tile.TileContext(nc) as tc:
        with (
            tc.tile_pool(name="persist", bufs=1) as persist,
            tc.tile_pool(name="wb_pool", bufs=4) as wbp,
        ):
            XCH = 4  # kb blocks per x chunk
            nxch = max(1, kba // XCH)
            xTs = [persist.tile([128, min(XCH, kba), tc_tokens], FP8,
                                name=f"xT{ci}")
                   for ci in range(nxch)]
            hT = persist.tile([128, nib, tc_tokens], FP8)
            cst = persist.tile([128, 8], F32)
            nc.scalar.dma_start(cst[:], cst_d.ap()[:, :])
            c_sig = cst[:, 0:1]   # 2^-(Ea+k) (sigmoid pre-scale)
            c_thr = cst[:, 1:2]   # 7 * 2^(Ea+k)
            c_nthr = cst[:, 2:3]  # -7 * 2^(Ea+k)
            c_h = cst[:, 3:4]     # 2^(m-2(Ea+k)) (h cast scale)
            c_out = cst[:, 4:5]   # 2^-(Eb+m) (down out scale)
            c_x = cst[:, 5:6]     # 2^k (x cast scale)
            c_nsig = cst[:, 6:7]  # -2^-(Ea+k) (exp pre-scale for sigmoid)

            # down-weight prefetch happens a few iterations into phase A
            wb_tiles = {}

            def ensure_wb(hm):
                if hm not in wb_tiles:
                    wt = wbp.tile([128, nib, 128], FP8, tag="wb")
                    nc.sync.dma_start(wt[:], wb_d.ap()[hm])
                    wb_tiles[hm] = wt
                return wb_tiles[hm]

            # ---- Phase A pools open early so the first weight pair can be
            # fetched ahead of the x chunks on the sync ring ----
            with (
                tc.tile_pool(name="wa_pool", bufs=10) as wap,
                tc.tile_pool(name="psA", bufs=8, space="PSUM") as psA,
                tc.tile_pool(name="swi", bufs=3) as swi,
                tc.tile_pool(name="ph0", bufs=min(3, nxch)) as p0,
            ):
                wa_pre = {}

                def wa_pair(i):
                    wg = wap.tile([128, kba, 128], FP8, tag="wa")
                    nc.sync.dma_start(wg[:], wa_d.ap()[i, 0])
                    wu = wap.tile([128, kba, 128], FP8, tag="wa")
                    nc.sync.dma_start(wu[:], wa_d.ap()[i, 1])
                    return wg, wu

                wa_pre[0] = wa_pair(0)
                wa_pre[1] = wa_pair(1)

                # HAM pre-warm: a burst of tiny matmuls on scratch data keeps
                # the PE clock-gate busy window alive while x loads, so the
                # first real matmuls run at 2.4 GHz instead of 1.2
                NWARM = 32
                warm = p0.tile([128, 2, tc_tokens], FP8, tag="warm")
                nc.vector.memset(warm[:], 0.0)
                wps = psA.tile([128, tc_tokens], F32, tag="ps", name="warm_ps")
                for wj in range(NWARM):
                    nc.tensor.matmul(
                        wps[:], lhsT=warm[:, :, 0:128], rhs=warm[:, :, :],
                        start=(wj == 0), stop=(wj == NWARM - 1), perf_mode=DR,
                    )

                # ---- Phase 0: load xT, scale by 2^k, cast f32 -> fp8 ----
                # x chunks alternate between the two HWDGE rings; casts
                # alternate between the scalar and vector engines
                xtmps = []
                for ci in range(nxch):
                    k0 = ci * XCH
                    kn = min(XCH, kba - k0)
                    xtmp = p0.tile([128, kn, tc_tokens], F32, tag="xt")
                    eng = nc.scalar if ci % 2 == 0 else nc.sync
                    eng.dma_start(xtmp[:], xt_d.ap()[:, k0 : k0 + kn, :])
                    xtmps.append(xtmp)
                for ci, xtmp in enumerate(xtmps):
                    if ci % 2 == 0:
                        nc.scalar.activation(xTs[ci][:], xtmp[:], AF.Copy, scale=c_x)
                    else:
                        nc.vector.tensor_scalar_mul(xTs[ci][:], xtmp[:], c_x)

                for i in range(nib):
                    wg, wu = wa_pre.pop(i, None) or wa_pair(i)
                    if i == 6:
                        ensure_wb(0)
                        ensure_wb(1)
                    psg = psA.tile([128, tc_tokens], F32, tag="ps")
                    psu = psA.tile([128, tc_tokens], F32, tag="ps")
                    for j in range(dka):
                        o = (2 * j) % XCH
                        nc.tensor.matmul(
                            psg[:], lhsT=wg[:, 2 * j : 2 * j + 2, :],
                            rhs=xTs[(2 * j) // XCH][:, o : o + 2, :],
                            start=(j == 0), stop=(j == dka - 1), perf_mode=DR,
                        )
                    for j in range(dka):
                        o = (2 * j) % XCH
                        nc.tensor.matmul(
                            psu[:], lhsT=wu[:, 2 * j : 2 * j + 2, :],
                            rhs=xTs[(2 * j) // XCH][:, o : o + 2, :],
                            start=(j == 0), stop=(j == dka - 1), perf_mode=DR,
                        )
                    # swiglu on [gate | up] tiles (cols = I-block i, free = tokens)
                    # gc' = min(psg, 7*2^Ea); true gate = psg*2^-Ea
                    gc = swi.tile([128, tc_tokens], F32, tag="gc")
                    nc.vector.tensor_scalar_min(gc[:], psg[:], c_thr)
                    uc = swi.tile([128, tc_tokens], F32, tag="uc")
                    nc.vector.tensor_scalar(
                        out=uc[:], in0=psu[:], scalar1=c_thr, scalar2=c_nthr,
                        op0=ALU.min, op1=ALU.max,
                    )
                    sg = swi.tile([128, tc_tokens], F32, tag="sg")
                    nc.scalar.activation(sg[:], gc[:], AF.Sigmoid, scale=c_sig)
                    m1 = swi.tile([128, tc_tokens], F32, tag="m1")
                    nc.vector.tensor_mul(m1[:], sg[:], gc[:])
                    t2 = swi.tile([128, tc_tokens], F32, tag="t2")
                    nc.vector.tensor_mul(t2[:], m1[:], uc[:])
                    # h = t2 * 2^-2Ea, cast to fp8 (exact vs reference qdq grid)
                    nc.scalar.activation(hT[:, i, :], t2[:], AF.Copy, scale=c_h)

            # ---- Phase B: down fp8 matmuls ----
            with (
                tc.tile_pool(name="psB", bufs=8, space="PSUM") as psB,
                tc.tile_pool(name="oev", bufs=4) as oev,
            ):
                for hm in range(nhb):
                    wt = ensure_wb(hm)
                    psd = psB.tile([128, tc_tokens], F32, tag="psd")
                    for j in range(dkb):
                        nc.tensor.matmul(
                            psd[:], lhsT=wt[:, 2 * j : 2 * j + 2, :],
                            rhs=hT[:, 2 * j : 2 * j + 2, :],
                            start=(j == 0), stop=(j == dkb - 1), perf_mode=DR,
                        )
                    ot = oev.tile([128, tc_tokens], F32, tag="ot")
                    nc.scalar.activation(ot[:], psd[:], AF.Copy, scale=c_out)
                    nc.sync.dma_start(out_d.ap()[hm], ot[:])

    if waitfix:
        from waitfix import split_multi_waits
        split_multi_waits(nc)
    return nc


# waitfix inlined so kernel.py stays self-contained
import sys as _sys
import types as _types

if "waitfix" not in _sys.modules:
    _wf = _types.ModuleType("waitfix")

    def _split_multi_waits(nc, limit: int = 1) -> int:
        n_split = 0
        f = nc.m.functions[0]
        for blk in f.blocks:
            insts = blk.instructions  # live list
            i = 0
            while i < len(insts):
                ins = insts[i]
                si = ins.sync_info
                if si is not None and len(si.on_wait) > limit:
                    waits = list(si.on_wait)
                    keep = waits[-limit:]
                    extra = waits[:-limit]
                    new_nops = []
                    for w in extra:
                        nop = mybir.InstNoOp(name=f"WSPLIT-{nc.next_id()}", ins=[], outs=[])
                        nop.engine = ins.engine
                        nop.sync_info = mybir.SyncInfo(on_wait=[w], on_update=[])
                        new_nops.append(nop)
                    ins.sync_info = mybir.SyncInfo(on_wait=keep, on_update=list(si.on_update))
                    for j, nop in enumerate(new_nops):
                        insts.insert(i + j, nop)
                    i += len(new_nops)
                    n_split += 1
                i += 1
        return n_split

    _wf.split_multi_waits = _split_multi_waits
    _sys.modules["waitfix"] = _wf


def _dequant(w, s, block=128):
    ob, ib = s.shape
    w4 = w.reshape(ob, block, ib, block) * s[:, None, :, None]
    return w4.reshape(ob * block, ib * block)


def _fold_fp8(wdeq):
    """Fold a dequantized weight matrix into fp8 with one global power-of-2
    scale: returns (w8, 2^-E) with w8 = fp8(wdeq * 2^E), |w8| <= 224."""
    absmax = float(np.abs(wdeq).max())
    if absmax == 0.0:
        return wdeq.astype(NP_FP8), 1.0
    E = int(np.floor(np.log2(224.0 / absmax)))
    E = max(min(E, 120), -120)
    w8 = np.clip(wdeq * np.float32(2.0**E), -240.0, 240.0).astype(NP_FP8)
    return w8, float(2.0**-E)


H_M = 2  # static power-of-2 pre-scale for the h cast (|h| <= 49 -> *4 <= 196)


def prep_weights(w_gate_up, s_gate_up, w_down, s_down, xmax, h=H, i_dim=I):
    """Host-side: dequantize (exact in f32), fold into fp8 with global
    power-of-2 scales, transpose into the [k_in_block, kb, col] layout.

    The x side gets a global pre-scale 2^k (k from xmax) and h a static 2^m
    so small values stay clear of the fp8 denormal floor; both are exact
    power-of-2 shifts that commute with the reference's qdq rounding and
    are undone in the eviction constants.
    """
    nib = i_dim // 128
    kba = h // 128
    nhb = h // 128

    wdeqA = _dequant(w_gate_up, s_gate_up)                # [2I, H] f32
    w8a, ca = _fold_fp8(wdeqA)
    ga = w8a[:i_dim].reshape(nib, 128, kba, 128).transpose(0, 3, 2, 1)
    ua = w8a[i_dim:].reshape(nib, 128, kba, 128).transpose(0, 3, 2, 1)
    wa = np.ascontiguousarray(np.stack([ga, ua], axis=1))  # [i, 2, k_in, kb, col]

    wdeqB = _dequant(w_down, s_down)                      # [H, I] f32
    w8b, cb = _fold_fp8(wdeqB)
    wb = np.ascontiguousarray(
        w8b.reshape(nhb, 128, nib, 128).transpose(0, 3, 2, 1)
    )

    k = int(np.floor(np.log2(224.0 / xmax))) if xmax > 0 else 0
    k = max(min(k, 120), -120)
    xk = float(2.0**k)
    caf = ca / xk                     # 2^-(Ea+k): psum-units -> true gate/up

    cst = np.zeros((128, 8), dtype=np.float32)
    cst[:, 0] = caf                   # sigmoid pre-scale
    cst[:, 1] = LIMIT / caf           # gate/up clip threshold in psum units
    cst[:, 2] = -LIMIT / caf
    cst[:, 3] = caf * caf * (2.0**H_M)  # h cast scale (stores h * 2^m)
    cst[:, 4] = cb / (2.0**H_M)       # down out scale
    cst[:, 5] = xk                    # x cast scale 2^k
    cst[:, 6] = -caf                  # exp pre-scale (sigmoid via exp)
    return wa, wb, cst


def prep_x(x):
    """x [T, H] f32 -> per-core [128, kba, TC] f32 transposed slices
    (partition dim first so one DMA covers many kb blocks)."""
    xTf = np.ascontiguousarray(x.T)                       # [H, T]
    outs = []
    for c in range(N_CORES):
        sl = xTf[:, c * TC : (c + 1) * TC].reshape(KBA, 128, TC)
        outs.append(np.ascontiguousarray(sl.transpose(1, 0, 2)))
    return outs


def make_in_maps(x, w_gate_up, s_gate_up, w_down, s_down):
    x = np.asarray(x, np.float32)
    wa, wb, cst = prep_weights(
        np.asarray(w_gate_up, np.float32), np.asarray(s_gate_up, np.float32),
        np.asarray(w_down, np.float32), np.asarray(s_down, np.float32),
        xmax=float(np.abs(x).max()),
    )
    xts = prep_x(x)
    return [
        {"xt": xts[c], "wa": wa, "wb": wb, "cst": cst}
        for c in range(N_CORES)
    ]


def unshard(results):
    """results[c]["out"] [NHB, 128, TC] f32 -> full [T, H]."""
    outs = []
    for c in range(N_CORES):
        o = results[c]["out"].reshape(H, TC)              # [hcol, token]
        outs.append(np.ascontiguousarray(o.T))            # [token, hcol]
    return np.concatenate(outs, axis=0)


_CACHE = {}


def kernel(x, w_gate_up, s_gate_up, w_down, s_down):
    if "nc" not in _CACHE:
        _CACHE["nc"] = build_nc()
    nc = _CACHE["nc"]
    in_maps = make_in_maps(x, w_gate_up, s_gate_up, w_down, s_down)
    out = None
    for _attempt in range(3):
        res = bass_utils.run_bass_kernel_spmd(
            nc, in_maps, core_ids=list(range(N_CORES))
        )
        out = unshard(res.results)
        if np.isfinite(out).all():
            break
        # transient transport corruption has been observed to inject fp8
        # NaN bit patterns; rerun rather than return garbage
    return out
